# revision 21
# baseline (speedup 1.0000x reference)
"""Trainium2 Bass kernel for the ClebschCombiningSingleUnrolled module.

The reference computes, for every (n, f) element independently,
    out[n, f, o] = sum_{i,j} W[o, i, j] * X1[n, f, i] * X2[n, f, j]
where W (5x7x7, 43 structural nonzeros) is a fixed linear function of the
7x7 `clebsch` input.  W is built on the host; the device kernel is a
memory-bound elementwise bilinear map, data-parallel over N across 8 cores.

Per-core device plan (fp16 intermediates, fp32 in/out):
  - DMA in interleaved [128, T*7] tiles (contiguous per partition)
  - ACT (scalar engine): deinterleave+downcast to per-channel fp16 planes
  - DVE: raw products (tensor_tensor, 2x mode), per-term scale
    (tensor_scalar, 4x mode, weight read from a [128,1] AP so the NEFF is
    independent of clebsch values), accumulate chains (tensor_tensor)
  - ACT: interleave accumulators into [128, T*5] fp32 output tile
  - DMA out
"""

import numpy as np

L1, L2, LAMBD = 3, 3, 2
SQRT2 = float(np.sqrt(2.0))
SQRT2_INV = float(1.0 / np.sqrt(2.0))

N_CORES = 8
P = 128          # SBUF partitions
T = 512          # (n*F+f) elements per partition per tile
CH = 7
OCH = 5
N_FULL, F_FULL = 50000, 200
N_SHARD = N_FULL // N_CORES              # 6250 rows per core
L_SHARD = N_SHARD * F_FULL               # 1,250,000 (n,f) elements per core
TILE_ELEMS = P * T                       # 65536 elements per tile
N_TILES = (L_SHARD + TILE_ELEMS - 1) // TILE_ELEMS   # 20 (last tile overlaps)

# ---------------------------------------------------------------------------
# Host-side math: build W[5,7,7] from clebsch
# ---------------------------------------------------------------------------

def _real_imag_np(X, l):
    res, ims = [], []
    for m in range(-l, l + 1):
        if m < 0:
            re = X[-m + l] * SQRT2_INV
            im = -X[m + l] * SQRT2_INV
        elif m == 0:
            re = X[l]
            im = np.zeros_like(X[l])
        else:
            sign = 1.0 if m % 2 == 0 else -1.0
            re = sign * X[m + l] * SQRT2_INV
            im = sign * X[-m + l] * SQRT2_INV
        res.append(re)
        ims.append(im)
    return np.stack(res), np.stack(ims)


def _combine_np(X1, X2, clebsch):
    X1 = np.swapaxes(X1, 0, 2)
    X2 = np.swapaxes(X2, 0, 2)
    x1_re, x1_im = _real_imag_np(X1, L1)
    x2_re, x2_im = _real_imag_np(X2, L2)
    result = [None] * (2 * LAMBD + 1)
    for mu in range(0, LAMBD + 1):
        m2s = np.arange(max(-L2, mu - L1), min(L2, mu + L1) + 1)
        i1 = (mu - m2s) + L1
        i2 = m2s + L2
        cg = clebsch[i1, i2]
        a_re, a_im = x1_re[i1], x1_im[i1]
        b_re, b_im = x2_re[i2], x2_im[i2]
        real_now = np.einsum('k,kfn->fn', cg, a_re * b_re - a_im * b_im)
        imag_now = np.einsum('k,kfn->fn', cg, a_re * b_im + a_im * b_re)
        if (L1 + L2 - LAMBD) % 2 == 1:
            imag_now, real_now = real_now, -imag_now
        if mu > 0:
            s = SQRT2 if mu % 2 == 0 else -SQRT2
            result[mu + LAMBD] = s * real_now
            result[-mu + LAMBD] = s * imag_now
        else:
            result[LAMBD] = real_now
    out = np.stack(result)
    return np.swapaxes(out, 0, 2)


def build_W(clebsch):
    """W[o,i,j] with out[...,o] = sum_ij W[o,i,j] X1[...,i] X2[...,j]."""
    W = np.zeros((OCH, CH, CH), dtype=np.float64)
    for i in range(CH):
        for j in range(CH):
            e1 = np.zeros((1, 1, CH)); e1[0, 0, i] = 1.0
            e2 = np.zeros((1, 1, CH)); e2[0, 0, j] = 1.0
            W[:, i, j] = _combine_np(e1, e2, np.asarray(clebsch, np.float64))[0, 0, :]
    return W


# Structural nonzero pattern (independent of clebsch values; verified by
# probing build_W with every basis clebsch).  TERMS[o] = list of (i, j).
TERMS = [
    [(0, 4), (1, 3), (2, 4), (2, 6), (3, 1), (4, 0), (4, 2), (6, 2)],
    [(0, 5), (1, 4), (1, 6), (2, 3), (2, 5), (3, 2), (4, 1), (5, 0), (5, 2), (6, 1)],
    [(0, 0), (1, 1), (2, 2), (3, 3), (4, 4), (5, 5), (6, 6)],
    [(0, 1), (1, 0), (1, 2), (2, 1), (3, 4), (4, 3), (4, 5), (5, 4), (5, 6), (6, 5)],
    [(0, 2), (2, 0), (2, 2), (3, 5), (4, 4), (4, 6), (5, 3), (6, 4)],
]
N_TERMS = sum(len(t) for t in TERMS)     # 43
W_PAD = 64                               # wcoef free-dim size (padded)

# x2-grouped rows: for each output o, group terms by the X1 channel i so the
# weighted X2 combination g = sum_j w*x2_j is built once per row, then a
# single product x1_i * g and one accumulate per row.
ROWS = []
for _terms in TERMS:
    _d = {}
    for (_i, _j) in _terms:
        _d.setdefault(_i, []).append(_j)
    ROWS.append(sorted(_d.items()))

# ---------------------------------------------------------------------------
# Bass program
# ---------------------------------------------------------------------------

# Engine assignment patterns (cycled per op category):
#   "V" = vector (DVE), "A" = scalar (ACT), "G" = gpsimd (Pool)
SCALE_PAT = ["A", "A", "G", "V", "G", "A"]
ADD_PAT = ["V", "V", "V", "G"]
CONV_PAT = ["A", "A", "V", "V"]


def build_bass(n_tiles=N_TILES, t_free=T, wmat=None):
    import concourse.bacc as bacc
    import concourse.tile as tile
    import concourse.mybir as mybir
    from contextlib import ExitStack

    f32 = mybir.dt.float32
    f16 = mybir.dt.float16
    Alu = mybir.AluOpType
    Act = mybir.ActivationFunctionType

    assert wmat is not None
    wmat = np.asarray(wmat, dtype=np.float64)

    nc = bacc.Bacc()
    x1d = nc.declare_dram_parameter("x1", [n_tiles, P, t_free * CH], f32, isOutput=False)
    x2d = nc.declare_dram_parameter("x2", [n_tiles, P, t_free * CH], f32, isOutput=False)
    outd = nc.declare_dram_parameter("out", [n_tiles, P, t_free * OCH], f32, isOutput=True)

    with tile.TileContext(nc) as tc, ExitStack() as ctx:
        const_pool = ctx.enter_context(tc.tile_pool(name="const", bufs=1))
        stage_pool = ctx.enter_context(tc.tile_pool(name="stage", bufs=2))
        plane_pool = ctx.enter_context(tc.tile_pool(name="plane", bufs=2))
        prod_pool = ctx.enter_context(tc.tile_pool(name="prod", bufs=8))
        sc_pool = ctx.enter_context(tc.tile_pool(name="sc", bufs=8))
        chain_pool = ctx.enter_context(tc.tile_pool(name="chain", bufs=8))
        ac_pool = ctx.enter_context(tc.tile_pool(name="ac", bufs=2))
        out_pool = ctx.enter_context(tc.tile_pool(name="ot", bufs=2))

        ncv = {"c": 0}

        def conv_to(dst, src):
            # contiguous fp32 -> fp16 downcast, engine per CONV_PAT
            eng = CONV_PAT[ncv["c"] % len(CONV_PAT)]
            ncv["c"] += 1
            if eng == "A":
                nc.scalar.copy(dst, src)
            else:
                nc.vector.tensor_copy(dst, src)

        for t in range(n_tiles):
            # host-side layout is already channel-deinterleaved:
            # [128, c, t] fp32 planes, contiguous per partition
            x1t = stage_pool.tile([P, CH, t_free], f32, tag="x1t")
            nc.sync.dma_start(x1t[:], x1d[t])
            x2t = stage_pool.tile([P, CH, t_free], f32, tag="x2t")
            nc.sync.dma_start(x2t[:], x2d[t])

            # downcast to fp16 planes (split into halves for engine balance)
            half = (CH * t_free) // 2
            x1p = plane_pool.tile([P, CH, t_free], f16, tag="x1p")
            x1pf = x1p[:].rearrange("p c t -> p (c t)")
            x1tf = x1t[:].rearrange("p c t -> p (c t)")
            conv_to(x1pf[:, :half], x1tf[:, :half])
            conv_to(x1pf[:, half:], x1tf[:, half:])
            x2p = plane_pool.tile([P, CH, t_free], f16, tag="x2p")
            x2pf = x2p[:].rearrange("p c t -> p (c t)")
            x2tf = x2t[:].rearrange("p c t -> p (c t)")
            conv_to(x2pf[:, :half], x2tf[:, :half])
            conv_to(x2pf[:, half:], x2tf[:, half:])

            ac = ac_pool.tile([P, OCH, t_free], f16, tag="ac")
            ns = {"s": 0, "a": 0}

            def scale_to(dst, src, w):
                eng = SCALE_PAT[ns["s"] % len(SCALE_PAT)]
                ns["s"] += 1
                if eng == "A":
                    nc.scalar.mul(dst, src, float(w))
                elif eng == "G":
                    nc.gpsimd.tensor_scalar(dst, src, float(w), None, op0=Alu.mult)
                else:
                    nc.vector.tensor_scalar(dst, src, float(w), None, op0=Alu.mult)

            def add_to(dst, a, b):
                eng = ADD_PAT[ns["a"] % len(ADD_PAT)]
                ns["a"] += 1
                e = nc.gpsimd if eng == "G" else nc.vector
                e.tensor_tensor(dst, a, b, Alu.add)

            for o in range(OCH):
                rows = ROWS[o]
                acc = None
                for ridx, (i, js) in enumerate(rows):
                    # g = sum_j w[o,i,j] * x2_j   (1 or 2 terms)
                    if len(js) == 1:
                        g = sc_pool.tile([P, t_free], f16, tag="sc",
                                         name=f"g_{t}_{o}_{ridx}")
                        scale_to(g[:], x2p[:, js[0], :], wmat[o, i, js[0]])
                        gap = g[:]
                    else:
                        sa = sc_pool.tile([P, t_free], f16, tag="sc",
                                          name=f"sa_{t}_{o}_{ridx}")
                        scale_to(sa[:], x2p[:, js[0], :], wmat[o, i, js[0]])
                        sb = sc_pool.tile([P, t_free], f16, tag="sc",
                                          name=f"sb_{t}_{o}_{ridx}")
                        scale_to(sb[:], x2p[:, js[1], :], wmat[o, i, js[1]])
                        g = sc_pool.tile([P, t_free], f16, tag="sc",
                                         name=f"g2_{t}_{o}_{ridx}")
                        add_to(g[:], sa[:], sb[:])
                        gap = g[:]
                    # product
                    prod = prod_pool.tile([P, t_free], f16, tag="prod",
                                          name=f"p_{t}_{o}_{ridx}")
                    nc.vector.tensor_tensor(prod[:], x1p[:, i, :], gap, Alu.mult)
                    # accumulate
                    if acc is None:
                        acc = prod[:]
                    else:
                        if ridx == len(rows) - 1:
                            dst = ac[:, o, :]
                        else:
                            ctile = chain_pool.tile([P, t_free], f16, tag="chain",
                                                    name=f"c_{t}_{o}_{ridx}")
                            dst = ctile[:]
                        add_to(dst, acc, prod[:])
                        acc = dst

            # interleave to fp32 [128, (t o)] and store
            ot = out_pool.tile([P, t_free * OCH], f32, tag="ot")
            nc.scalar.activation(
                ot[:].rearrange("p (t o) -> p t o", o=OCH),
                ac[:].transpose([0, 2, 1]),
                Act.Copy,
            )
            nc.sync.dma_start(outd[t], ot[:])

    nc.finalize()
    return nc


# ---------------------------------------------------------------------------
# Host wrapper
# ---------------------------------------------------------------------------

def _tile_shard(arr2d):
    """arr2d: [L_SHARD, CH]; -> [N_TILES, P, CH*T] channel-plane layout
    (per tile/partition: CH contiguous planes of T elements), with the last
    tile overlapping the end of the shard."""
    ch = arr2d.shape[1]
    tiles = np.empty((N_TILES, P, ch * T), dtype=arr2d.dtype)
    full = (L_SHARD // TILE_ELEMS)                       # 19 full tiles
    body = arr2d[: full * TILE_ELEMS].reshape(full, P, T, ch)
    tiles[:full] = body.transpose(0, 1, 3, 2).reshape(full, P, ch * T)
    tail = arr2d[L_SHARD - TILE_ELEMS:].reshape(1, P, T, ch)
    tiles[full:] = tail.transpose(0, 1, 3, 2).reshape(1, P, ch * T)
    return tiles


def kernel(X1, X2, clebsch):
    from concourse.bass_utils import run_bass_kernel_spmd

    X1 = np.ascontiguousarray(np.asarray(X1, dtype=np.float32))
    X2 = np.ascontiguousarray(np.asarray(X2, dtype=np.float32))
    clebsch = np.asarray(clebsch, dtype=np.float32)

    W = build_W(clebsch)
    nc = build_bass(wmat=W)

    in_maps = []
    for c in range(N_CORES):
        x1s = X1[c * N_SHARD:(c + 1) * N_SHARD].reshape(-1, CH)
        x2s = X2[c * N_SHARD:(c + 1) * N_SHARD].reshape(-1, CH)
        in_maps.append({
            "x1": _tile_shard(x1s),
            "x2": _tile_shard(x2s),
        })

    res = run_bass_kernel_spmd(nc, in_maps, list(range(N_CORES)))
    outs = res.results

    full = L_SHARD // TILE_ELEMS
    out = np.empty((N_FULL, F_FULL, OCH), dtype=np.float32)
    for c in range(N_CORES):
        o_t = outs[c]["out"]                 # [N_TILES, P, T*OCH]
        flat = np.empty((L_SHARD * OCH,), dtype=np.float32)
        flat[: full * TILE_ELEMS * OCH] = o_t[:full].reshape(-1)
        tail_start = (L_SHARD - TILE_ELEMS) * OCH
        flat[tail_start:] = o_t[full:].reshape(-1)
        out[c * N_SHARD:(c + 1) * N_SHARD] = flat.reshape(N_SHARD, F_FULL, OCH)
    return out


# revision 23
# speedup vs baseline: 26.6939x; 26.6939x over previous
"""Trainium2 Bass kernel for the ClebschCombiningSingleUnrolled module.

The reference computes, for every (n, f) element independently,
    out[n, f, o] = sum_{i,j} W[o, i, j] * X1[n, f, i] * X2[n, f, j]
where W (5x7x7, 43 structural nonzeros) is a fixed linear function of the
7x7 `clebsch` input.  W is built on the host; the device kernel is a
memory-bound elementwise bilinear map, data-parallel over N across 8 cores.

Per-core device plan (fp16 intermediates, fp32 in/out):
  - DMA in interleaved [128, T*7] tiles (contiguous per partition)
  - ACT (scalar engine): deinterleave+downcast to per-channel fp16 planes
  - DVE: raw products (tensor_tensor, 2x mode), per-term scale
    (tensor_scalar, 4x mode, weight read from a [128,1] AP so the NEFF is
    independent of clebsch values), accumulate chains (tensor_tensor)
  - ACT: interleave accumulators into [128, T*5] fp32 output tile
  - DMA out
"""

import numpy as np

L1, L2, LAMBD = 3, 3, 2
SQRT2 = float(np.sqrt(2.0))
SQRT2_INV = float(1.0 / np.sqrt(2.0))

N_CORES = 8
P = 128          # SBUF partitions
T = 512          # (n*F+f) elements per partition per tile
CH = 7
OCH = 5
N_FULL, F_FULL = 50000, 200
N_SHARD = N_FULL // N_CORES              # 6250 rows per core
L_SHARD = N_SHARD * F_FULL               # 1,250,000 (n,f) elements per core
TILE_ELEMS = P * T                       # 65536 elements per tile
N_TILES = (L_SHARD + TILE_ELEMS - 1) // TILE_ELEMS   # 20 (last tile overlaps)

# ---------------------------------------------------------------------------
# Host-side math: build W[5,7,7] from clebsch
# ---------------------------------------------------------------------------

def _real_imag_np(X, l):
    res, ims = [], []
    for m in range(-l, l + 1):
        if m < 0:
            re = X[-m + l] * SQRT2_INV
            im = -X[m + l] * SQRT2_INV
        elif m == 0:
            re = X[l]
            im = np.zeros_like(X[l])
        else:
            sign = 1.0 if m % 2 == 0 else -1.0
            re = sign * X[m + l] * SQRT2_INV
            im = sign * X[-m + l] * SQRT2_INV
        res.append(re)
        ims.append(im)
    return np.stack(res), np.stack(ims)


def _combine_np(X1, X2, clebsch):
    X1 = np.swapaxes(X1, 0, 2)
    X2 = np.swapaxes(X2, 0, 2)
    x1_re, x1_im = _real_imag_np(X1, L1)
    x2_re, x2_im = _real_imag_np(X2, L2)
    result = [None] * (2 * LAMBD + 1)
    for mu in range(0, LAMBD + 1):
        m2s = np.arange(max(-L2, mu - L1), min(L2, mu + L1) + 1)
        i1 = (mu - m2s) + L1
        i2 = m2s + L2
        cg = clebsch[i1, i2]
        a_re, a_im = x1_re[i1], x1_im[i1]
        b_re, b_im = x2_re[i2], x2_im[i2]
        real_now = np.einsum('k,kfn->fn', cg, a_re * b_re - a_im * b_im)
        imag_now = np.einsum('k,kfn->fn', cg, a_re * b_im + a_im * b_re)
        if (L1 + L2 - LAMBD) % 2 == 1:
            imag_now, real_now = real_now, -imag_now
        if mu > 0:
            s = SQRT2 if mu % 2 == 0 else -SQRT2
            result[mu + LAMBD] = s * real_now
            result[-mu + LAMBD] = s * imag_now
        else:
            result[LAMBD] = real_now
    out = np.stack(result)
    return np.swapaxes(out, 0, 2)


def build_W(clebsch):
    """W[o,i,j] with out[...,o] = sum_ij W[o,i,j] X1[...,i] X2[...,j]."""
    W = np.zeros((OCH, CH, CH), dtype=np.float64)
    for i in range(CH):
        for j in range(CH):
            e1 = np.zeros((1, 1, CH)); e1[0, 0, i] = 1.0
            e2 = np.zeros((1, 1, CH)); e2[0, 0, j] = 1.0
            W[:, i, j] = _combine_np(e1, e2, np.asarray(clebsch, np.float64))[0, 0, :]
    return W


# Structural nonzero pattern (independent of clebsch values; verified by
# probing build_W with every basis clebsch).  TERMS[o] = list of (i, j).
TERMS = [
    [(0, 4), (1, 3), (2, 4), (2, 6), (3, 1), (4, 0), (4, 2), (6, 2)],
    [(0, 5), (1, 4), (1, 6), (2, 3), (2, 5), (3, 2), (4, 1), (5, 0), (5, 2), (6, 1)],
    [(0, 0), (1, 1), (2, 2), (3, 3), (4, 4), (5, 5), (6, 6)],
    [(0, 1), (1, 0), (1, 2), (2, 1), (3, 4), (4, 3), (4, 5), (5, 4), (5, 6), (6, 5)],
    [(0, 2), (2, 0), (2, 2), (3, 5), (4, 4), (4, 6), (5, 3), (6, 4)],
]
N_TERMS = sum(len(t) for t in TERMS)     # 43
W_PAD = 64                               # wcoef free-dim size (padded)

# x2-grouped rows: for each output o, group terms by the X1 channel i so the
# weighted X2 combination g = sum_j w*x2_j is built once per row, then a
# single product x1_i * g and one accumulate per row.
ROWS = []
for _terms in TERMS:
    _d = {}
    for (_i, _j) in _terms:
        _d.setdefault(_i, []).append(_j)
    ROWS.append(sorted(_d.items()))

# ---------------------------------------------------------------------------
# Bass program
# ---------------------------------------------------------------------------

# Engine assignment patterns (cycled per op category):
#   "V" = vector (DVE), "A" = scalar (ACT), "G" = gpsimd (Pool)
SCALE_PAT = ["A", "A", "G", "V", "G", "A"]
ADD_PAT = ["V", "V", "V", "G"]
CONV_PAT = ["A", "A", "V", "V"]


def build_bass(n_tiles=N_TILES, t_free=T, wmat=None, repeat=None):
    import concourse.bacc as bacc
    import concourse.tile as tile
    import concourse.mybir as mybir
    from contextlib import ExitStack, nullcontext

    f32 = mybir.dt.float32
    f16 = mybir.dt.float16
    Alu = mybir.AluOpType
    Act = mybir.ActivationFunctionType

    assert wmat is not None
    wmat = np.asarray(wmat, dtype=np.float64)

    nc = bacc.Bacc()
    x1d = nc.declare_dram_parameter("x1", [n_tiles, P, t_free * CH], f32, isOutput=False)
    x2d = nc.declare_dram_parameter("x2", [n_tiles, P, t_free * CH], f32, isOutput=False)
    outd = nc.declare_dram_parameter("out", [n_tiles, P, t_free * OCH], f32, isOutput=True)

    with tile.TileContext(nc) as tc, ExitStack() as ctx:
        const_pool = ctx.enter_context(tc.tile_pool(name="const", bufs=1))
        stage_pool = ctx.enter_context(tc.tile_pool(name="stage", bufs=2))
        plane_pool = ctx.enter_context(tc.tile_pool(name="plane", bufs=2))
        prod_pool = ctx.enter_context(tc.tile_pool(name="prod", bufs=8))
        sc_pool = ctx.enter_context(tc.tile_pool(name="sc", bufs=8))
        chain_pool = ctx.enter_context(tc.tile_pool(name="chain", bufs=8))
        ac_pool = ctx.enter_context(tc.tile_pool(name="ac", bufs=2))
        out_pool = ctx.enter_context(tc.tile_pool(name="ot", bufs=2))

        ncv = {"c": 0}

        def conv_to(dst, src):
            # contiguous fp32 -> fp16 downcast, engine per CONV_PAT
            eng = CONV_PAT[ncv["c"] % len(CONV_PAT)]
            ncv["c"] += 1
            if eng == "A":
                nc.scalar.copy(dst, src)
            else:
                nc.vector.tensor_copy(dst, src)

        rep_ctx = tc.For_i(0, repeat, 1) if repeat else nullcontext()
        with rep_ctx:
         for t in range(n_tiles):
            # host-side layout is already channel-deinterleaved:
            # [128, c, t] fp32 planes, contiguous per partition
            x1t = stage_pool.tile([P, CH, t_free], f32, tag="x1t")
            nc.sync.dma_start(x1t[:], x1d[t])
            x2t = stage_pool.tile([P, CH, t_free], f32, tag="x2t")
            nc.sync.dma_start(x2t[:], x2d[t])

            # downcast to fp16 planes (split into halves for engine balance)
            half = (CH * t_free) // 2
            x1p = plane_pool.tile([P, CH, t_free], f16, tag="x1p")
            x1pf = x1p[:].rearrange("p c t -> p (c t)")
            x1tf = x1t[:].rearrange("p c t -> p (c t)")
            conv_to(x1pf[:, :half], x1tf[:, :half])
            conv_to(x1pf[:, half:], x1tf[:, half:])
            x2p = plane_pool.tile([P, CH, t_free], f16, tag="x2p")
            x2pf = x2p[:].rearrange("p c t -> p (c t)")
            x2tf = x2t[:].rearrange("p c t -> p (c t)")
            conv_to(x2pf[:, :half], x2tf[:, :half])
            conv_to(x2pf[:, half:], x2tf[:, half:])

            ac = ac_pool.tile([P, OCH, t_free], f16, tag="ac")
            ns = {"s": 0, "a": 0}

            def scale_to(dst, src, w):
                eng = SCALE_PAT[ns["s"] % len(SCALE_PAT)]
                ns["s"] += 1
                if eng == "A":
                    nc.scalar.mul(dst, src, float(w))
                elif eng == "G":
                    nc.gpsimd.tensor_scalar(dst, src, float(w), None, op0=Alu.mult)
                else:
                    nc.vector.tensor_scalar(dst, src, float(w), None, op0=Alu.mult)

            def add_to(dst, a, b):
                eng = ADD_PAT[ns["a"] % len(ADD_PAT)]
                ns["a"] += 1
                e = nc.gpsimd if eng == "G" else nc.vector
                e.tensor_tensor(dst, a, b, Alu.add)

            for o in range(OCH):
                rows = ROWS[o]
                acc = None
                for ridx, (i, js) in enumerate(rows):
                    # g = sum_j w[o,i,j] * x2_j   (1 or 2 terms)
                    if len(js) == 1:
                        g = sc_pool.tile([P, t_free], f16, tag="sc",
                                         name=f"g_{t}_{o}_{ridx}")
                        scale_to(g[:], x2p[:, js[0], :], wmat[o, i, js[0]])
                        gap = g[:]
                    else:
                        sa = sc_pool.tile([P, t_free], f16, tag="sc",
                                          name=f"sa_{t}_{o}_{ridx}")
                        scale_to(sa[:], x2p[:, js[0], :], wmat[o, i, js[0]])
                        sb = sc_pool.tile([P, t_free], f16, tag="sc",
                                          name=f"sb_{t}_{o}_{ridx}")
                        scale_to(sb[:], x2p[:, js[1], :], wmat[o, i, js[1]])
                        g = sc_pool.tile([P, t_free], f16, tag="sc",
                                         name=f"g2_{t}_{o}_{ridx}")
                        add_to(g[:], sa[:], sb[:])
                        gap = g[:]
                    # product
                    prod = prod_pool.tile([P, t_free], f16, tag="prod",
                                          name=f"p_{t}_{o}_{ridx}")
                    nc.vector.tensor_tensor(prod[:], x1p[:, i, :], gap, Alu.mult)
                    # accumulate
                    if acc is None:
                        acc = prod[:]
                    else:
                        if ridx == len(rows) - 1:
                            dst = ac[:, o, :]
                        else:
                            ctile = chain_pool.tile([P, t_free], f16, tag="chain",
                                                    name=f"c_{t}_{o}_{ridx}")
                            dst = ctile[:]
                        add_to(dst, acc, prod[:])
                        acc = dst

            # interleave to fp32 [128, (t o)] and store
            ot = out_pool.tile([P, t_free * OCH], f32, tag="ot")
            nc.scalar.activation(
                ot[:].rearrange("p (t o) -> p t o", o=OCH),
                ac[:].transpose([0, 2, 1]),
                Act.Copy,
            )
            nc.sync.dma_start(outd[t], ot[:])

    nc.finalize()
    return nc


# ---------------------------------------------------------------------------
# Host wrapper
# ---------------------------------------------------------------------------

def _tile_shard(arr2d):
    """arr2d: [L_SHARD, CH]; -> [N_TILES, P, CH*T] channel-plane layout
    (per tile/partition: CH contiguous planes of T elements), with the last
    tile overlapping the end of the shard."""
    ch = arr2d.shape[1]
    tiles = np.empty((N_TILES, P, ch * T), dtype=arr2d.dtype)
    full = (L_SHARD // TILE_ELEMS)                       # 19 full tiles
    body = arr2d[: full * TILE_ELEMS].reshape(full, P, T, ch)
    tiles[:full] = body.transpose(0, 1, 3, 2).reshape(full, P, ch * T)
    tail = arr2d[L_SHARD - TILE_ELEMS:].reshape(1, P, T, ch)
    tiles[full:] = tail.transpose(0, 1, 3, 2).reshape(1, P, ch * T)
    return tiles


def kernel(X1, X2, clebsch):
    from concourse.bass_utils import run_bass_kernel_spmd

    X1 = np.ascontiguousarray(np.asarray(X1, dtype=np.float32))
    X2 = np.ascontiguousarray(np.asarray(X2, dtype=np.float32))
    clebsch = np.asarray(clebsch, dtype=np.float32)

    W = build_W(clebsch)
    nc = build_bass(wmat=W)

    in_maps = []
    for c in range(N_CORES):
        x1s = X1[c * N_SHARD:(c + 1) * N_SHARD].reshape(-1, CH)
        x2s = X2[c * N_SHARD:(c + 1) * N_SHARD].reshape(-1, CH)
        in_maps.append({
            "x1": _tile_shard(x1s),
            "x2": _tile_shard(x2s),
        })

    res = run_bass_kernel_spmd(nc, in_maps, list(range(N_CORES)))
    outs = res.results

    full = L_SHARD // TILE_ELEMS
    out = np.empty((N_FULL, F_FULL, OCH), dtype=np.float32)
    for c in range(N_CORES):
        o_t = outs[c]["out"]                 # [N_TILES, P, T*OCH]
        flat = np.empty((L_SHARD * OCH,), dtype=np.float32)
        flat[: full * TILE_ELEMS * OCH] = o_t[:full].reshape(-1)
        tail_start = (L_SHARD - TILE_ELEMS) * OCH
        flat[tail_start:] = o_t[full:].reshape(-1)
        out[c * N_SHARD:(c + 1) * N_SHARD] = flat.reshape(N_SHARD, F_FULL, OCH)
    return out


# revision 25
# speedup vs baseline: 95.7074x; 3.5854x over previous
"""Trainium2 Bass kernel for the ClebschCombiningSingleUnrolled module.

The reference computes, for every (n, f) element independently,
    out[n, f, o] = sum_{i,j} W[o, i, j] * X1[n, f, i] * X2[n, f, j]
where W (5x7x7, 43 structural nonzeros) is a fixed linear function of the
7x7 `clebsch` input.  W is built on the host; the device kernel is a
memory-bound elementwise bilinear map, data-parallel over N across 8 cores.

Per-core device plan (fp16 intermediates, fp32 in/out):
  - host pre-shards N across 8 cores and lays tiles out as per-channel
    planes ([128 partitions, 7 channels, T elems], fp32)
  - DMA in; downcast fp32->fp16 (split between ACT and DVE)
  - per output o and X1-channel row: g = sum_j w*x2_j (tensor_scalar on
    ACT/DVE + tensor_tensor add), product x1_i*g (DVE tensor_tensor, 2x
    fp16 mode), accumulate chain (DVE)
  - ACT: interleave the 5 accumulators into [128, T*5] fp32 output tile
  - DMA out.  gpsimd is used for nothing: its fp16 tensor ops measured
    ~10x slower than the cost model on hardware
"""

import numpy as np

L1, L2, LAMBD = 3, 3, 2
SQRT2 = float(np.sqrt(2.0))
SQRT2_INV = float(1.0 / np.sqrt(2.0))

N_CORES = 8
P = 128          # SBUF partitions
T = 512          # (n*F+f) elements per partition per tile
CH = 7
OCH = 5
N_FULL, F_FULL = 50000, 200
N_SHARD = N_FULL // N_CORES              # 6250 rows per core
L_SHARD = N_SHARD * F_FULL               # 1,250,000 (n,f) elements per core
TILE_ELEMS = P * T                       # 65536 elements per tile
N_TILES = (L_SHARD + TILE_ELEMS - 1) // TILE_ELEMS   # 20 (last tile overlaps)

# ---------------------------------------------------------------------------
# Host-side math: build W[5,7,7] from clebsch
# ---------------------------------------------------------------------------

def _real_imag_np(X, l):
    res, ims = [], []
    for m in range(-l, l + 1):
        if m < 0:
            re = X[-m + l] * SQRT2_INV
            im = -X[m + l] * SQRT2_INV
        elif m == 0:
            re = X[l]
            im = np.zeros_like(X[l])
        else:
            sign = 1.0 if m % 2 == 0 else -1.0
            re = sign * X[m + l] * SQRT2_INV
            im = sign * X[-m + l] * SQRT2_INV
        res.append(re)
        ims.append(im)
    return np.stack(res), np.stack(ims)


def _combine_np(X1, X2, clebsch):
    X1 = np.swapaxes(X1, 0, 2)
    X2 = np.swapaxes(X2, 0, 2)
    x1_re, x1_im = _real_imag_np(X1, L1)
    x2_re, x2_im = _real_imag_np(X2, L2)
    result = [None] * (2 * LAMBD + 1)
    for mu in range(0, LAMBD + 1):
        m2s = np.arange(max(-L2, mu - L1), min(L2, mu + L1) + 1)
        i1 = (mu - m2s) + L1
        i2 = m2s + L2
        cg = clebsch[i1, i2]
        a_re, a_im = x1_re[i1], x1_im[i1]
        b_re, b_im = x2_re[i2], x2_im[i2]
        real_now = np.einsum('k,kfn->fn', cg, a_re * b_re - a_im * b_im)
        imag_now = np.einsum('k,kfn->fn', cg, a_re * b_im + a_im * b_re)
        if (L1 + L2 - LAMBD) % 2 == 1:
            imag_now, real_now = real_now, -imag_now
        if mu > 0:
            s = SQRT2 if mu % 2 == 0 else -SQRT2
            result[mu + LAMBD] = s * real_now
            result[-mu + LAMBD] = s * imag_now
        else:
            result[LAMBD] = real_now
    out = np.stack(result)
    return np.swapaxes(out, 0, 2)


def build_W(clebsch):
    """W[o,i,j] with out[...,o] = sum_ij W[o,i,j] X1[...,i] X2[...,j]."""
    W = np.zeros((OCH, CH, CH), dtype=np.float64)
    for i in range(CH):
        for j in range(CH):
            e1 = np.zeros((1, 1, CH)); e1[0, 0, i] = 1.0
            e2 = np.zeros((1, 1, CH)); e2[0, 0, j] = 1.0
            W[:, i, j] = _combine_np(e1, e2, np.asarray(clebsch, np.float64))[0, 0, :]
    return W


# Structural nonzero pattern (independent of clebsch values; verified by
# probing build_W with every basis clebsch).  TERMS[o] = list of (i, j).
TERMS = [
    [(0, 4), (1, 3), (2, 4), (2, 6), (3, 1), (4, 0), (4, 2), (6, 2)],
    [(0, 5), (1, 4), (1, 6), (2, 3), (2, 5), (3, 2), (4, 1), (5, 0), (5, 2), (6, 1)],
    [(0, 0), (1, 1), (2, 2), (3, 3), (4, 4), (5, 5), (6, 6)],
    [(0, 1), (1, 0), (1, 2), (2, 1), (3, 4), (4, 3), (4, 5), (5, 4), (5, 6), (6, 5)],
    [(0, 2), (2, 0), (2, 2), (3, 5), (4, 4), (4, 6), (5, 3), (6, 4)],
]
N_TERMS = sum(len(t) for t in TERMS)     # 43
W_PAD = 64                               # wcoef free-dim size (padded)

# x2-grouped rows: for each output o, group terms by the X1 channel i so the
# weighted X2 combination g = sum_j w*x2_j is built once per row, then a
# single product x1_i * g and one accumulate per row.
ROWS = []
for _terms in TERMS:
    _d = {}
    for (_i, _j) in _terms:
        _d.setdefault(_i, []).append(_j)
    ROWS.append(sorted(_d.items()))

# ---------------------------------------------------------------------------
# Bass program
# ---------------------------------------------------------------------------

# Engine assignment patterns (cycled per op category):
#   "V" = vector (DVE), "A" = scalar (ACT), "G" = gpsimd (Pool)
# NOTE: gpsimd ("G") is deliberately absent — its fp16 tensor ucode measured
# ~10x slower than the cost model on real TRN2 hardware.
SCALE_PAT = ["A", "A", "V", "V", "A"]
ADD_PAT = ["V"]
CONV_PAT = ["A", "A", "V", "V"]


def build_bass(n_tiles=N_TILES, t_free=T, wmat=None, repeat=None):
    import concourse.bacc as bacc
    import concourse.tile as tile
    import concourse.mybir as mybir
    from contextlib import ExitStack, nullcontext

    f32 = mybir.dt.float32
    f16 = mybir.dt.float16
    Alu = mybir.AluOpType
    Act = mybir.ActivationFunctionType

    assert wmat is not None
    wmat = np.asarray(wmat, dtype=np.float64)

    nc = bacc.Bacc()
    x1d = nc.declare_dram_parameter("x1", [n_tiles, P, t_free * CH], f32, isOutput=False)
    x2d = nc.declare_dram_parameter("x2", [n_tiles, P, t_free * CH], f32, isOutput=False)
    outd = nc.declare_dram_parameter("out", [n_tiles, P, t_free * OCH], f32, isOutput=True)

    with tile.TileContext(nc) as tc, ExitStack() as ctx:
        const_pool = ctx.enter_context(tc.tile_pool(name="const", bufs=1))
        stage_pool = ctx.enter_context(tc.tile_pool(name="stage", bufs=2))
        plane_pool = ctx.enter_context(tc.tile_pool(name="plane", bufs=2))
        prod_pool = ctx.enter_context(tc.tile_pool(name="prod", bufs=8))
        sc_pool = ctx.enter_context(tc.tile_pool(name="sc", bufs=8))
        chain_pool = ctx.enter_context(tc.tile_pool(name="chain", bufs=8))
        ac_pool = ctx.enter_context(tc.tile_pool(name="ac", bufs=2))
        out_pool = ctx.enter_context(tc.tile_pool(name="ot", bufs=2))

        ncv = {"c": 0}

        def conv_to(dst, src):
            # contiguous fp32 -> fp16 downcast, engine per CONV_PAT
            eng = CONV_PAT[ncv["c"] % len(CONV_PAT)]
            ncv["c"] += 1
            if eng == "A":
                nc.scalar.copy(dst, src)
            else:
                nc.vector.tensor_copy(dst, src)

        rep_ctx = tc.For_i(0, repeat, 1) if repeat else nullcontext()
        with rep_ctx:
         for t in range(n_tiles):
            # host-side layout is already channel-deinterleaved:
            # [128, c, t] fp32 planes, contiguous per partition
            x1t = stage_pool.tile([P, CH, t_free], f32, tag="x1t")
            nc.sync.dma_start(x1t[:], x1d[t])
            x2t = stage_pool.tile([P, CH, t_free], f32, tag="x2t")
            nc.sync.dma_start(x2t[:], x2d[t])

            # downcast to fp16 planes (split into halves for engine balance)
            half = (CH * t_free) // 2
            x1p = plane_pool.tile([P, CH, t_free], f16, tag="x1p")
            x1pf = x1p[:].rearrange("p c t -> p (c t)")
            x1tf = x1t[:].rearrange("p c t -> p (c t)")
            conv_to(x1pf[:, :half], x1tf[:, :half])
            conv_to(x1pf[:, half:], x1tf[:, half:])
            x2p = plane_pool.tile([P, CH, t_free], f16, tag="x2p")
            x2pf = x2p[:].rearrange("p c t -> p (c t)")
            x2tf = x2t[:].rearrange("p c t -> p (c t)")
            conv_to(x2pf[:, :half], x2tf[:, :half])
            conv_to(x2pf[:, half:], x2tf[:, half:])

            ac = ac_pool.tile([P, OCH, t_free], f16, tag="ac")
            ns = {"s": 0, "a": 0}

            def scale_to(dst, src, w):
                eng = SCALE_PAT[ns["s"] % len(SCALE_PAT)]
                ns["s"] += 1
                if eng == "A":
                    nc.scalar.mul(dst, src, float(w))
                elif eng == "G":
                    nc.gpsimd.tensor_scalar(dst, src, float(w), None, op0=Alu.mult)
                else:
                    nc.vector.tensor_scalar(dst, src, float(w), None, op0=Alu.mult)

            def add_to(dst, a, b):
                eng = ADD_PAT[ns["a"] % len(ADD_PAT)]
                ns["a"] += 1
                e = nc.gpsimd if eng == "G" else nc.vector
                e.tensor_tensor(dst, a, b, Alu.add)

            for o in range(OCH):
                rows = ROWS[o]
                acc = None
                for ridx, (i, js) in enumerate(rows):
                    # g = sum_j w[o,i,j] * x2_j   (1 or 2 terms)
                    if len(js) == 1:
                        g = sc_pool.tile([P, t_free], f16, tag="sc",
                                         name=f"g_{t}_{o}_{ridx}")
                        scale_to(g[:], x2p[:, js[0], :], wmat[o, i, js[0]])
                        gap = g[:]
                    else:
                        sa = sc_pool.tile([P, t_free], f16, tag="sc",
                                          name=f"sa_{t}_{o}_{ridx}")
                        scale_to(sa[:], x2p[:, js[0], :], wmat[o, i, js[0]])
                        sb = sc_pool.tile([P, t_free], f16, tag="sc",
                                          name=f"sb_{t}_{o}_{ridx}")
                        scale_to(sb[:], x2p[:, js[1], :], wmat[o, i, js[1]])
                        g = sc_pool.tile([P, t_free], f16, tag="sc",
                                         name=f"g2_{t}_{o}_{ridx}")
                        add_to(g[:], sa[:], sb[:])
                        gap = g[:]
                    # product
                    prod = prod_pool.tile([P, t_free], f16, tag="prod",
                                          name=f"p_{t}_{o}_{ridx}")
                    nc.vector.tensor_tensor(prod[:], x1p[:, i, :], gap, Alu.mult)
                    # accumulate
                    if acc is None:
                        acc = prod[:]
                    else:
                        if ridx == len(rows) - 1:
                            dst = ac[:, o, :]
                        else:
                            ctile = chain_pool.tile([P, t_free], f16, tag="chain",
                                                    name=f"c_{t}_{o}_{ridx}")
                            dst = ctile[:]
                        add_to(dst, acc, prod[:])
                        acc = dst

            # interleave to fp32 [128, (t o)] and store
            ot = out_pool.tile([P, t_free * OCH], f32, tag="ot")
            nc.scalar.activation(
                ot[:].rearrange("p (t o) -> p t o", o=OCH),
                ac[:].transpose([0, 2, 1]),
                Act.Copy,
            )
            nc.sync.dma_start(outd[t], ot[:])

    nc.finalize()
    return nc


# ---------------------------------------------------------------------------
# Host wrapper
# ---------------------------------------------------------------------------

def _tile_shard(arr2d):
    """arr2d: [L_SHARD, CH]; -> [N_TILES, P, CH*T] channel-plane layout
    (per tile/partition: CH contiguous planes of T elements), with the last
    tile overlapping the end of the shard."""
    ch = arr2d.shape[1]
    tiles = np.empty((N_TILES, P, ch * T), dtype=arr2d.dtype)
    full = (L_SHARD // TILE_ELEMS)                       # 19 full tiles
    body = arr2d[: full * TILE_ELEMS].reshape(full, P, T, ch)
    tiles[:full] = body.transpose(0, 1, 3, 2).reshape(full, P, ch * T)
    tail = arr2d[L_SHARD - TILE_ELEMS:].reshape(1, P, T, ch)
    tiles[full:] = tail.transpose(0, 1, 3, 2).reshape(1, P, ch * T)
    return tiles


def kernel(X1, X2, clebsch):
    from concourse.bass_utils import run_bass_kernel_spmd

    X1 = np.ascontiguousarray(np.asarray(X1, dtype=np.float32))
    X2 = np.ascontiguousarray(np.asarray(X2, dtype=np.float32))
    clebsch = np.asarray(clebsch, dtype=np.float32)

    W = build_W(clebsch)
    nc = build_bass(wmat=W)

    in_maps = []
    for c in range(N_CORES):
        x1s = X1[c * N_SHARD:(c + 1) * N_SHARD].reshape(-1, CH)
        x2s = X2[c * N_SHARD:(c + 1) * N_SHARD].reshape(-1, CH)
        in_maps.append({
            "x1": _tile_shard(x1s),
            "x2": _tile_shard(x2s),
        })

    res = run_bass_kernel_spmd(nc, in_maps, list(range(N_CORES)))
    outs = res.results

    full = L_SHARD // TILE_ELEMS
    out = np.empty((N_FULL, F_FULL, OCH), dtype=np.float32)
    for c in range(N_CORES):
        o_t = outs[c]["out"]                 # [N_TILES, P, T*OCH]
        flat = np.empty((L_SHARD * OCH,), dtype=np.float32)
        flat[: full * TILE_ELEMS * OCH] = o_t[:full].reshape(-1)
        tail_start = (L_SHARD - TILE_ELEMS) * OCH
        flat[tail_start:] = o_t[full:].reshape(-1)
        out[c * N_SHARD:(c + 1) * N_SHARD] = flat.reshape(N_SHARD, F_FULL, OCH)
    return out
